# revision 24
# baseline (speedup 1.0000x reference)
"""Trainium2 Bass kernel for the conv->softmax->NLL loss (nn_ARM_71665824301873).

Math (per reference):
  h = Conv2d(1->256, 3x3, pad=1)(x) + b          # (N, 256, 64, 64)
  prob = softmax over classes; idx = floor(x*255)
  loss = mean_n [ sum_pix ( LSE(h) - h[idx] ) ]   # clamp in reference never
                                                  # fires for this regime
Strategy: pure data parallel, 8 images per core (N=64 over 8 cores).
Per core:
  - conv as K=10 matmul (9 taps + ones-row for bias), classes on PSUM
    partitions, pixels streamed on the free dim. Tap rows are a host-side
    re-layout of x (pure data movement), shipped as one DRAM tensor.
  - exp on ScalarE (PSUM->SBUF bf16); per-pixel class-sum via "indicator"
    matmuls (lhsT one-hot column -> image-quarter sums land on partitions
    0..3), staged to SBUF by VectorE; one Ln+accumulate at the end.
  - gathered term h[idx]: GPSIMD ap_gather pulls W[:, idx]/b[idx] from a
    256-entry table per pixel; fused multiply+reduce against the tap rows
    on VectorE (scalar_tensor_tensor accum_out). Runs up front, fully
    overlapped with the conv/softmax pipeline.
  - host sums the 8 per-core partials and divides by N (data-parallel mean).
"""

import numpy as np
import ml_dtypes

BF16 = ml_dtypes.bfloat16

N_CORES = 8
IMGS = 8          # images per core
H = Wd = 64
D = H * Wd        # 4096 pixels per image
K = 256           # classes
PW = 66           # padded image width for tap shifts
IMG_PAD = PW * PW  # 4356

# taps: (dy, dx) in conv output terms h[i,j] += W[t] * x[i+dy, j+dx]
TAPS = [(dy, dx) for dy in (-1, 0, 1) for dx in (-1, 0, 1)]  # t = 3*(dy+1)+(dx+1)

_COMPILED = {}


def _build_nc(idx_shift: float):
    from contextlib import ExitStack

    import concourse.bass as bass
    import concourse.bacc as bacc
    import concourse.tile as tile
    import concourse.mybir as mybir

    f32 = mybir.dt.float32
    bf16 = mybir.dt.bfloat16
    i16 = mybir.dt.int16
    AF = mybir.ActivationFunctionType
    ALU = mybir.AluOpType

    nc = bacc.Bacc(None)
    taps_d = nc.declare_dram_parameter("taps", [128, 2 * IMG_PAD], bf16,
                                       isOutput=False)
    xw_d = nc.declare_dram_parameter("xw", [128, D // 16], f32, isOutput=False)
    wq_d = nc.declare_dram_parameter("wq", [128, 256], bf16, isOutput=False)
    wtab_d = nc.declare_dram_parameter("wtab", [128, 256], f32, isOutput=False)
    ones4_d = nc.declare_dram_parameter("ones4", [128, 16], bf16, isOutput=False)
    out_d = nc.declare_dram_parameter("out", [1, 1], f32, isOutput=True)

    with tile.TileContext(nc) as tc, ExitStack() as ctx:
        pers = ctx.enter_context(tc.tile_pool(name="pers", bufs=1))
        expp = ctx.enter_context(tc.tile_pool(name="expp", bufs=6))
        hps = ctx.enter_context(tc.tile_pool(name="hps", bufs=2, space="PSUM"))
        seps = ctx.enter_context(tc.tile_pool(name="seps", bufs=2, space="PSUM"))

        # ---------------- persistent SBUF tiles ----------------
        patch = pers.tile([128, 2 * IMG_PAD], bf16)   # tap rows (host layout)
        wqs = pers.tile([128, 256], bf16)             # conv lhsT replicas
        wtab = pers.tile([128, 256], f32)             # gather table
        ones4 = pers.tile([128, 16], bf16)            # indicator lhsT blocks
        xw = pers.tile([128, D // 16], f32)           # x wrapped for idx
        idx = pers.tile([128, D // 16], i16)
        wsel = pers.tile([128, D], f32)               # gathered table rows
        selprod = pers.tile([128, D], f32)            # stt elementwise scratch
        ses = pers.tile([128, IMGS * 1024], f32)      # sumexp staging [4, 8k]
        lse_s = pers.tile([128, IMGS * 1024], bf16)   # Ln main-out scratch
        acc_lse = pers.tile([128, 1], f32)
        acc_hsel = pers.tile([128, 1], f32)
        fin = pers.tile([128, 1], f32)
        onescol = pers.tile([128, 1], f32)
        negcol = pers.tile([128, 1], f32)

        # ---------------- loads ----------------
        nc.sync.dma_start(wqs[:, :], wq_d[:, :])
        nc.sync.dma_start(xw[:, :], xw_d[:, :])
        nc.scalar.dma_start(wtab[:, :], wtab_d[:, :])
        nc.scalar.dma_start(ones4[:, :], ones4_d[:, :])
        # one strip (2 imgs of taps) per DMA queue issue, spread over engines
        for q in range(4):
            eng = (nc.sync, nc.scalar, nc.gpsimd, nc.sync)[q]
            eng.dma_start(patch[32 * q:32 * q + 32, :],
                          taps_d[32 * q:32 * q + 32, :])

        nc.vector.memset(acc_hsel[:, :], 0.0)
        nc.vector.memset(onescol[:, :], 1.0)
        nc.vector.memset(negcol[:, :], -1.0)

        # ---------------- idx + gather + gathered-dot (front-loaded) -------
        nc.vector.tensor_scalar(idx[:, :], xw[:, :], 255.0, idx_shift,
                                ALU.mult, ALU.add)
        nc.gpsimd.ap_gather(
            wsel[:, :].rearrange("p (n d) -> p n d", d=1),
            wtab[:, :].rearrange("p (n d) -> p n d", d=1),
            idx[:, :],
            channels=128, num_elems=256, d=1, num_idxs=D,
        )
        for q in range(4):
            pv = patch[32 * q:32 * q + 26, 0:IMG_PAD].rearrange(
                "p (r c) -> p r c", c=PW)[:, 1:65, 1:65]
            wv = wsel[32 * q:32 * q + 26, :].rearrange("p (r c) -> p r c", c=Wd)
            nc.vector.scalar_tensor_tensor(
                selprod[32 * q:32 * q + 26, :].rearrange("p (r c) -> p r c", c=Wd),
                pv, 1.0, wv,
                ALU.mult, ALU.mult,
                accum_out=acc_hsel[32 * q:32 * q + 26, 0:1],
            )

        # ---------------- main pipeline over 32 image-quarters -------------
        # stage 1: conv matmuls -> PSUM; stage 2: exp (ACT) -> SBUF bf16;
        # stage 3: indicator matmuls -> per-quarter sumexp on partitions 0..3
        # stage 4: VE copy PSUM->SBUF staging.
        # Indicator matmuls are emitted one quarter late so the PE never
        # stalls in-order behind exp.
        pending = None  # (g, s, ehs, se)

        def flush_pending():
            nonlocal pending
            if pending is None:
                return
            g, s, ehs, se = pending
            for u in range(2):
                for hh in range(2):
                    nc.tensor.matmul(
                        se[0:4, u * 512:u * 512 + 512],
                        ones4[:, 4 * s:4 * s + 4],
                        ehs[hh][:, u * 512:(u + 1) * 512],
                        start=(s == 0 and hh == 0),
                        stop=(s == 3 and hh == 1),
                        tile_position=(0, 0),
                    )
            if s == 3:
                # stage the finished image's [4, 1024] sumexp to SBUF
                nc.vector.tensor_copy(ses[0:4, g * 1024:(g + 1) * 1024],
                                      se[0:4, :])
            pending = None

        se = None
        for g in range(IMGS):
            q = g // 2
            off = (g % 2) * IMG_PAD
            conv_v = patch[32 * q:32 * q + 10, off:off + IMG_PAD].rearrange(
                "p (r c) -> p r c", c=PW)[:, 1:65, 1:65]
            se_g = seps.tile([128, 1024], f32, tag="se")
            for s in range(4):           # four 1024-pixel quarters
                ehs = []
                for hh in range(2):      # class halves
                    hp = hps.tile([128, 1024], f32, tag="h")
                    for sub in range(2):  # 512-px (8-row) matmuls
                        r0 = s * 16 + sub * 8
                        nc.tensor.matmul(
                            hp[:, sub * 512:(sub + 1) * 512],
                            wqs[32 * q:32 * q + 10, hh * 128:(hh + 1) * 128],
                            conv_v[:, r0:r0 + 8, :],
                            start=True, stop=True,
                            tile_position=(32 * q, 0),
                        )
                    eh = expp.tile([128, 1024], bf16, tag="eh")
                    nc.scalar.activation(eh[:, :], hp[:, :], AF.Exp)
                    ehs.append(eh)
                flush_pending()
                pending = (g, s, ehs, se_g)
        flush_pending()

        # ---------------- LSE: one Ln + accumulate over everything ---------
        nc.scalar.activation(lse_s[0:4, :], ses[0:4, :], AF.Ln,
                             accum_out=acc_lse[0:4, 0:1])

        # ---------------- final combine ----------------
        # loss_partial = sum(acc_lse) - sum(acc_hsel), accumulated in one
        # PSUM cell via a +1 column and a -1 column.
        red_a = seps.tile([128, 1024], f32, tag="se")
        nc.tensor.matmul(red_a[0:1, 0:1], onescol[0:4, 0:1],
                         acc_lse[0:4, 0:1], start=True, stop=False,
                         tile_position=(0, 0))
        nc.tensor.matmul(red_a[0:1, 0:1], negcol[:, 0:1],
                         acc_hsel[:, :], start=False, stop=True,
                         tile_position=(0, 0), skip_group_check=True)
        nc.vector.tensor_copy(fin[0:1, 0:1], red_a[0:1, 0:1])
        nc.sync.dma_start(out_d[:, :], fin[0:1, 0:1])

    nc.finalize()
    return nc


def _host_inputs(x, W, b):
    """Per-core input maps (host-side re-layout only, no arithmetic on x
    beyond dtype conversion)."""
    x = np.ascontiguousarray(np.asarray(x, dtype=np.float32).reshape(64, H, Wd))
    W = np.asarray(W, dtype=np.float32).reshape(K, 3, 3)
    b = np.asarray(b, dtype=np.float32)

    # conv lhsT replicas: strip 32Q+t rows, col block hh -> W[t, class]
    wq = np.zeros((128, 256), dtype=BF16)
    wtab = np.zeros((128, 256), dtype=np.float32)
    for t, (dy, dx) in enumerate(TAPS):
        wrow = W[:, 1 + dy, 1 + dx]
        for q in range(4):
            wq[32 * q + t, :] = wrow.astype(BF16)
        for g in range(8):
            wtab[16 * g + t, :] = wrow
    for q in range(4):
        wq[32 * q + 9, :] = b.astype(BF16)
    for g in range(8):
        wtab[16 * g + 9, :] = b

    ones4 = np.zeros((128, 16), dtype=BF16)
    for qg in range(4):
        ones4[:, 4 * qg + qg] = BF16(1.0)

    in_maps = []
    for c in range(N_CORES):
        xs = x[c * IMGS:(c + 1) * IMGS]                      # (8, 64, 64) f32
        # tap rows in the exact SBUF layout the kernel expects:
        #   row 32Q+t, free [n*IMG_PAD ...]: img 2Q+n shifted by tap t
        #   row 32Q+16+t, free [0:IMG_PAD]: img 2Q+1 shifted by tap t (copy)
        #   row 32Q+9 / +16+9: "ones" rows (1 at real pixels)
        xb = xs.astype(BF16)
        shifted = np.zeros((IMGS, 10, PW, PW), dtype=BF16)   # [img, tap, 66, 66]
        for t, (dy, dx) in enumerate(TAPS):
            r0, r1 = max(0, dy), min(H, H + dy)              # src rows
            c0, c1 = max(0, dx), min(Wd, Wd + dx)
            dr0 = 1 + (r0 - dy)
            dc0 = 1 + (c0 - dx)
            shifted[:, t, dr0:dr0 + (r1 - r0), dc0:dc0 + (c1 - c0)] = \
                xb[:, r0:r1, c0:c1]
        shifted[:, 9, 1:65, 1:65] = BF16(1.0)
        taps = np.zeros((128, 2 * IMG_PAD), dtype=BF16)
        for qq in range(4):
            for t in range(10):
                taps[32 * qq + t, 0:IMG_PAD] = shifted[2 * qq, t].reshape(-1)
                taps[32 * qq + t, IMG_PAD:] = shifted[2 * qq + 1, t].reshape(-1)
                taps[32 * qq + 16 + t, 0:IMG_PAD] = \
                    shifted[2 * qq + 1, t].reshape(-1)
        xw = np.ascontiguousarray(
            xs.reshape(IMGS, D // 16, 16).transpose(0, 2, 1).reshape(128, D // 16)
        ).astype(np.float32)
        in_maps.append({
            "taps": taps,
            "xw": xw,
            "wq": wq,
            "wtab": wtab,
            "ones4": ones4,
        })
    return in_maps


def kernel(x, W, b):
    from concourse.bass_utils import run_bass_kernel_spmd

    key = "main"
    if key not in _COMPILED:
        _COMPILED[key] = _build_nc(idx_shift=-0.5)
    nc = _COMPILED[key]
    in_maps = _host_inputs(x, W, b)
    res = run_bass_kernel_spmd(nc, in_maps, core_ids=list(range(N_CORES)))
    total = np.float64(0.0)
    for r in res.results:
        total += np.float64(r["out"].reshape(-1)[0])
    return np.float32(total / 64.0)


# revision 25
# speedup vs baseline: 1.6728x; 1.6728x over previous
"""Trainium2 Bass kernel for the conv->softmax->NLL loss (nn_ARM_71665824301873).

Math (per reference):
  h = Conv2d(1->256, 3x3, pad=1)(x) + b          # (N, 256, 64, 64)
  prob = softmax over classes; idx = floor(x*255)
  loss = mean_n [ sum_pix ( LSE(h) - h[idx] ) ]   # clamp in reference never
                                                  # fires for this regime
Strategy: pure data parallel, 8 images per core (N=64 over 8 cores).
Per core:
  - conv as K=10 matmul (9 taps + ones-row for bias), classes on PSUM
    partitions, pixels streamed on the free dim. Tap rows are a host-side
    re-layout of x (pure data movement), shipped as one DRAM tensor.
  - exp on ScalarE (PSUM->SBUF bf16); per-pixel class-sum via "indicator"
    matmuls (lhsT one-hot column -> image-quarter sums land on partitions
    0..3), staged to SBUF by VectorE; one Ln+accumulate at the end.
  - gathered term h[idx]: GPSIMD ap_gather pulls W[:, idx]/b[idx] from a
    256-entry table per pixel; fused multiply+reduce against the tap rows
    on VectorE (scalar_tensor_tensor accum_out). Runs up front, fully
    overlapped with the conv/softmax pipeline.
  - host sums the 8 per-core partials and divides by N (data-parallel mean).
"""

import numpy as np
import ml_dtypes

BF16 = ml_dtypes.bfloat16

N_CORES = 8
IMGS = 8          # images per core
H = Wd = 64
D = H * Wd        # 4096 pixels per image
K = 256           # classes
PW = 66           # padded image width for tap shifts
IMG_PAD = PW * PW  # 4356

# taps: (dy, dx) in conv output terms h[i,j] += W[t] * x[i+dy, j+dx]
TAPS = [(dy, dx) for dy in (-1, 0, 1) for dx in (-1, 0, 1)]  # t = 3*(dy+1)+(dx+1)

_COMPILED = {}


def _build_nc(idx_shift: float):
    from contextlib import ExitStack

    import concourse.bass as bass
    import concourse.bacc as bacc
    import concourse.tile as tile
    import concourse.mybir as mybir

    f32 = mybir.dt.float32
    bf16 = mybir.dt.bfloat16
    i16 = mybir.dt.int16
    AF = mybir.ActivationFunctionType
    ALU = mybir.AluOpType

    nc = bacc.Bacc(None)
    taps_d = nc.declare_dram_parameter("taps", [128, 2 * IMG_PAD], bf16,
                                       isOutput=False)
    xw_d = nc.declare_dram_parameter("xw", [128, D // 16], f32, isOutput=False)
    wq_d = nc.declare_dram_parameter("wq", [128, 256], bf16, isOutput=False)
    wtab_d = nc.declare_dram_parameter("wtab", [128, 256], f32, isOutput=False)
    ones32_d = nc.declare_dram_parameter("ones32", [128, 1024], bf16,
                                          isOutput=False)
    out_d = nc.declare_dram_parameter("out", [1, 1], f32, isOutput=True)

    with tile.TileContext(nc) as tc, ExitStack() as ctx:
        pers = ctx.enter_context(tc.tile_pool(name="pers", bufs=1))
        expp = ctx.enter_context(tc.tile_pool(name="expp", bufs=6))
        hps = ctx.enter_context(tc.tile_pool(name="hps", bufs=3, space="PSUM"))
        seps = ctx.enter_context(tc.tile_pool(name="seps", bufs=1, space="PSUM"))

        # ---------------- persistent SBUF tiles ----------------
        patch = pers.tile([128, 2 * IMG_PAD], bf16)   # tap rows (host layout)
        wqs = pers.tile([128, 256], bf16)             # conv lhsT replicas
        wtab = pers.tile([128, 256], f32)             # gather table
        ones32 = pers.tile([128, 1024], bf16)         # indicator lhsT blocks
        xw = pers.tile([128, D // 16], f32)           # x wrapped for idx
        idx = pers.tile([128, D // 16], i16)
        wsel = pers.tile([128, D], f32)               # gathered table rows
        selprod = pers.tile([128, D], f32)            # stt elementwise scratch
        lse_s = pers.tile([128, 1024], bf16)          # Ln main-out scratch
        acc_lse = pers.tile([128, 1], f32)
        acc_hsel = pers.tile([128, 1], f32)
        fin = pers.tile([128, 1], f32)
        onescol = pers.tile([128, 1], f32)
        negcol = pers.tile([128, 1], f32)

        # ---------------- loads ----------------
        nc.sync.dma_start(wqs[:, :], wq_d[:, :])
        nc.sync.dma_start(xw[:, :], xw_d[:, :])
        nc.scalar.dma_start(wtab[:, :], wtab_d[:, :])
        nc.scalar.dma_start(ones32[:, :], ones32_d[:, :])
        # half-strip tap DMAs spread across the two HWDGE queues
        for hq in range(8):
            eng = (nc.sync, nc.scalar)[hq % 2]
            eng.dma_start(patch[16 * hq:16 * hq + 16, :],
                          taps_d[16 * hq:16 * hq + 16, :])

        nc.vector.memset(acc_hsel[:, :], 0.0)
        nc.vector.memset(onescol[:, :], 1.0)
        nc.vector.memset(negcol[:, :], -1.0)

        # ---------------- idx + gather + gathered-dot (front-loaded) -------
        nc.vector.tensor_scalar(idx[:, :], xw[:, :], 255.0, idx_shift,
                                ALU.mult, ALU.add)
        nc.gpsimd.ap_gather(
            wsel[:, :].rearrange("p (n d) -> p n d", d=1),
            wtab[:, :].rearrange("p (n d) -> p n d", d=1),
            idx[:, :],
            channels=128, num_elems=256, d=1, num_idxs=D,
        )
        for q in range(4):
            pv = patch[32 * q:32 * q + 26, 0:IMG_PAD].rearrange(
                "p (r c) -> p r c", c=PW)[:, 1:65, 1:65]
            wv = wsel[32 * q:32 * q + 26, :].rearrange("p (r c) -> p r c", c=Wd)
            nc.vector.scalar_tensor_tensor(
                selprod[32 * q:32 * q + 26, :].rearrange("p (r c) -> p r c", c=Wd),
                pv, 1.0, wv,
                ALU.mult, ALU.mult,
                accum_out=acc_hsel[32 * q:32 * q + 26, 0:1],
            )

        # ---------------- main pipeline over 32 image-quarters -------------
        # stage 1: conv matmuls -> PSUM; stage 2: exp (ACT) -> SBUF bf16;
        # stage 3: indicator matmuls -> per-quarter sumexp on partitions 0..3
        # stage 4: VE copy PSUM->SBUF staging.
        # Indicator matmuls are emitted one quarter late so the PE never
        # stalls in-order behind exp.
        # one PSUM tile accumulates all 32 (img, quarter) sumexp rows:
        # img g quarter s -> partition 4g+s (indicator column in ones32)
        se = seps.tile([128, 1024], f32, tag="se")
        pending = None  # (k, ehs)

        def flush_pending(last=False):
            nonlocal pending
            if pending is None:
                return
            k, ehs = pending
            for u in range(2):
                for hh in range(2):
                    nc.tensor.matmul(
                        se[0:32, u * 512:u * 512 + 512],
                        ones32[:, 32 * k:32 * k + 32],
                        ehs[hh][:, u * 512:(u + 1) * 512],
                        start=(k == 0 and hh == 0),
                        stop=(k == 31 and hh == 1),
                        tile_position=(0, 0),
                        skip_group_check=True,
                    )
            pending = None

        for g in range(IMGS):
            q = g // 2
            off = (g % 2) * IMG_PAD
            conv_v = patch[32 * q:32 * q + 10, off:off + IMG_PAD].rearrange(
                "p (r c) -> p r c", c=PW)[:, 1:65, 1:65]
            for s in range(4):           # four 1024-pixel quarters
                ehs = []
                for hh in range(2):      # class halves
                    hp = hps.tile([128, 1024], f32, tag="h")
                    for sub in range(2):  # 512-px (8-row) matmuls
                        r0 = s * 16 + sub * 8
                        nc.tensor.matmul(
                            hp[:, sub * 512:(sub + 1) * 512],
                            wqs[32 * q:32 * q + 10, hh * 128:(hh + 1) * 128],
                            conv_v[:, r0:r0 + 8, :],
                            start=True, stop=True,
                            tile_position=(32 * q, 0),
                        )
                    eh = expp.tile([128, 1024], bf16, tag="eh")
                    nc.scalar.activation(eh[:, :], hp[:, :], AF.Exp)
                    ehs.append(eh)
                flush_pending()
                pending = (4 * g + s, ehs)
        flush_pending(last=True)

        # ---------------- LSE: one Ln + accumulate, straight from PSUM -----
        nc.scalar.activation(lse_s[0:32, :], se[0:32, :], AF.Ln,
                             accum_out=acc_lse[0:32, 0:1])

        # ---------------- final combine ----------------
        # loss_partial = sum(acc_lse) - sum(acc_hsel), accumulated in one
        # PSUM cell via a +1 column and a -1 column.
        nc.tensor.matmul(se[0:1, 0:1], onescol[0:32, 0:1],
                         acc_lse[0:32, 0:1], start=True, stop=False,
                         tile_position=(0, 0), skip_group_check=True)
        nc.tensor.matmul(se[0:1, 0:1], negcol[:, 0:1],
                         acc_hsel[:, :], start=False, stop=True,
                         tile_position=(0, 0), skip_group_check=True)
        nc.vector.tensor_copy(fin[0:1, 0:1], se[0:1, 0:1])
        nc.sync.dma_start(out_d[:, :], fin[0:1, 0:1])

    nc.finalize()
    return nc


def _host_inputs(x, W, b):
    """Per-core input maps (host-side re-layout only, no arithmetic on x
    beyond dtype conversion)."""
    x = np.ascontiguousarray(np.asarray(x, dtype=np.float32).reshape(64, H, Wd))
    W = np.asarray(W, dtype=np.float32).reshape(K, 3, 3)
    b = np.asarray(b, dtype=np.float32)

    # conv lhsT replicas: strip 32Q+t rows, col block hh -> W[t, class]
    wq = np.zeros((128, 256), dtype=BF16)
    wtab = np.zeros((128, 256), dtype=np.float32)
    for t, (dy, dx) in enumerate(TAPS):
        wrow = W[:, 1 + dy, 1 + dx]
        for q in range(4):
            wq[32 * q + t, :] = wrow.astype(BF16)
        for g in range(8):
            wtab[16 * g + t, :] = wrow
    for q in range(4):
        wq[32 * q + 9, :] = b.astype(BF16)
    for g in range(8):
        wtab[16 * g + 9, :] = b

    ones32 = np.zeros((128, 1024), dtype=BF16)
    for k in range(32):
        ones32[:, 32 * k + k] = BF16(1.0)

    in_maps = []
    for c in range(N_CORES):
        xs = x[c * IMGS:(c + 1) * IMGS]                      # (8, 64, 64) f32
        # tap rows in the exact SBUF layout the kernel expects:
        #   row 32Q+t, free [n*IMG_PAD ...]: img 2Q+n shifted by tap t
        #   row 32Q+16+t, free [0:IMG_PAD]: img 2Q+1 shifted by tap t (copy)
        #   row 32Q+9 / +16+9: "ones" rows (1 at real pixels)
        xb = xs.astype(BF16)
        shifted = np.zeros((IMGS, 10, PW, PW), dtype=BF16)   # [img, tap, 66, 66]
        for t, (dy, dx) in enumerate(TAPS):
            r0, r1 = max(0, dy), min(H, H + dy)              # src rows
            c0, c1 = max(0, dx), min(Wd, Wd + dx)
            dr0 = 1 + (r0 - dy)
            dc0 = 1 + (c0 - dx)
            shifted[:, t, dr0:dr0 + (r1 - r0), dc0:dc0 + (c1 - c0)] = \
                xb[:, r0:r1, c0:c1]
        shifted[:, 9, 1:65, 1:65] = BF16(1.0)
        taps = np.zeros((128, 2 * IMG_PAD), dtype=BF16)
        for qq in range(4):
            for t in range(10):
                taps[32 * qq + t, 0:IMG_PAD] = shifted[2 * qq, t].reshape(-1)
                taps[32 * qq + t, IMG_PAD:] = shifted[2 * qq + 1, t].reshape(-1)
                taps[32 * qq + 16 + t, 0:IMG_PAD] = \
                    shifted[2 * qq + 1, t].reshape(-1)
        xw = np.ascontiguousarray(
            xs.reshape(IMGS, D // 16, 16).transpose(0, 2, 1).reshape(128, D // 16)
        ).astype(np.float32)
        in_maps.append({
            "taps": taps,
            "xw": xw,
            "wq": wq,
            "wtab": wtab,
            "ones32": ones32,
        })
    return in_maps


def kernel(x, W, b):
    from concourse.bass_utils import run_bass_kernel_spmd

    key = "main"
    if key not in _COMPILED:
        _COMPILED[key] = _build_nc(idx_shift=-0.5)
    nc = _COMPILED[key]
    in_maps = _host_inputs(x, W, b)
    res = run_bass_kernel_spmd(nc, in_maps, core_ids=list(range(N_CORES)))
    total = np.float64(0.0)
    for r in res.results:
        total += np.float64(r["out"].reshape(-1)[0])
    return np.float32(total / 64.0)


# revision 41
# speedup vs baseline: 1.8850x; 1.1268x over previous
"""Trainium2 Bass kernel for the conv->softmax->NLL loss (nn_ARM_71665824301873).

Math (per reference):
  h = Conv2d(1->256, 3x3, pad=1)(x) + b          # (N, 256, 64, 64)
  prob = softmax over classes; idx = floor(x*255)
  loss = mean_n [ sum_pix ( LSE(h) - h[idx] ) ]   # reference's prob-clamp
                                                  # never fires in this regime
Strategy: pure data parallel, 8 images per core (N=64 over 8 cores).
Per core:
  - conv as K=10 matmul (9 taps + ones-row for bias): tap rows are a host
    re-layout of x (pure data movement); images run four at a time so the
    conv occupies all four PE row-groups concurrently.
  - exp on ScalarE (PSUM->SBUF bf16); per-pixel class-sums via "indicator"
    matmuls spread across all four PE col-groups; one PSUM tile accumulates
    all 32 (img, quarter) sumexp rows; Ln+accumulate straight from PSUM.
  - gathered term h[idx]: GPSIMD ap_gather pulls W[:, idx]/b[idx] rows from
    a 256-entry table; fused multiply+reduce against the tap rows on
    VectorE (scalar_tensor_tensor accum_out).
  - host sums the 8 per-core partials and divides by N (data-parallel mean).

Note: the Pool engine's wait_ge on this fleet sleeps unconditionally and
only wakes on semaphore updates that arrive strictly after it sleeps
(missed edges are rescued by a ~112us timeout). The gather is therefore
given spaced "heartbeat" ticks on its awaited semaphore.
"""

import numpy as np
import ml_dtypes

BF16 = ml_dtypes.bfloat16

N_CORES = 8
IMGS = 8          # images per core
H = Wd = 64
D = H * Wd        # 4096 pixels per image
K = 256           # classes
PW = 66           # padded image width for tap shifts
IMG_PAD = PW * PW  # 4356

# taps: (dy, dx) in conv output terms h[i,j] += W[t] * x[i+dy, j+dx]
TAPS = [(dy, dx) for dy in (-1, 0, 1) for dx in (-1, 0, 1)]  # t = 3*(dy+1)+(dx+1)

_COMPILED = {}


def _build_nc(idx_shift: float):
    from contextlib import ExitStack

    import concourse.bacc as bacc
    import concourse.tile as tile
    import concourse.mybir as mybir

    f32 = mybir.dt.float32
    bf16 = mybir.dt.bfloat16
    i16 = mybir.dt.int16
    AF = mybir.ActivationFunctionType
    ALU = mybir.AluOpType

    nc = bacc.Bacc(None)
    taps_d = nc.declare_dram_parameter("taps", [128, 2 * IMG_PAD], bf16,
                                       isOutput=False)
    xw_d = nc.declare_dram_parameter("xw", [128, D // 16], f32, isOutput=False)
    wq_d = nc.declare_dram_parameter("wq", [128, 256], bf16, isOutput=False)
    wtab_d = nc.declare_dram_parameter("wtab", [128, 256], f32, isOutput=False)
    ones32_d = nc.declare_dram_parameter("ones32", [128, 1024], bf16,
                                         isOutput=False)
    out_d = nc.declare_dram_parameter("out", [1, 1], f32, isOutput=True)

    with tile.TileContext(nc) as tc, ExitStack() as ctx:
        pers = ctx.enter_context(tc.tile_pool(name="pers", bufs=1))
        expp = ctx.enter_context(tc.tile_pool(name="expp", bufs=8))
        hps = ctx.enter_context(tc.tile_pool(name="hps", bufs=3, space="PSUM"))
        seps = ctx.enter_context(tc.tile_pool(name="seps", bufs=1, space="PSUM"))

        # ---------------- persistent SBUF tiles ----------------
        patch = pers.tile([128, 2 * IMG_PAD], bf16)   # tap rows (host layout)
        wqs = pers.tile([128, 256], bf16)             # conv lhsT replicas
        wtab = pers.tile([128, 256], f32)             # gather table (DMA)
        wtab2 = pers.tile([128, 256], f32)            # gather table (VE copy)
        ones32 = pers.tile([128, 1024], bf16)         # indicator lhsT blocks
        xw = pers.tile([128, D // 16], f32)           # x wrapped for idx
        idx = pers.tile([128, D // 16], i16)
        wsel = pers.tile([128, D], f32)               # gathered table rows
        selprod = pers.tile([128, D], bf16)           # stt elementwise scratch
        lse_s = pers.tile([128, 1024], bf16)          # Ln main-out scratch
        acc_lse = pers.tile([128, 1], f32)
        acc_hsel = pers.tile([128, 1], f32)
        fin = pers.tile([128, 1], f32)
        onescol = pers.tile([128, 1], f32)
        negcol = pers.tile([128, 1], f32)
        zsrc = pers.tile([128, 16], f32)
        hbsp = pers.tile([128, 256], f32)
        hbz0 = pers.tile([128, 16], f32)
        hbz1 = pers.tile([128, 16], f32)
        hbz2 = pers.tile([128, 16], f32)
        hbz3 = pers.tile([128, 16], f32)
        hbc0 = pers.tile([128, 16], f32)
        hbc1 = pers.tile([128, 16], f32)
        hbc2 = pers.tile([128, 16], f32)
        hbc3 = pers.tile([128, 16], f32)
        hbz = [hbz0, hbz1, hbz2, hbz3]
        hbc = [hbc0, hbc1, hbc2, hbc3]

        # ---------------- loads ----------------
        nc.sync.dma_start(wtab[:, :], wtab_d[:, :])
        nc.scalar.dma_start(xw[:, :], xw_d[:, :])

        nc.vector.memset(acc_hsel[:, :], 0.0)
        nc.vector.memset(acc_lse[:, :], 0.0)
        nc.vector.memset(onescol[:, :], 1.0)
        nc.vector.memset(negcol[:, :], -1.0)
        nc.vector.memset(zsrc[:, :], 0.0)

        # ---------------- idx + gather (front-loaded) ----------------------
        nc.vector.tensor_scalar(idx[:, :], xw[:, :], 255.0, idx_shift,
                                ALU.mult, ALU.add)
        nc.vector.tensor_copy(wtab2[:, :], wtab[:, :])
        # heartbeat ticks (see module docstring): spaced DVE-semaphore
        # updates after the gather's wait goes to sleep, each consumed by a
        # tiny ScalarE copy so Tile emits an increment for it.
        for j in range(4):
            # isolate the final tick ~8us after the previous one so the
            # Pool's post-attempt re-arm window cannot swallow it
            for _ in range(25 if j == 3 else 3):
                nc.vector.tensor_copy(hbsp[:, :], wtab2[:, :])
            nc.vector.tensor_copy(hbz[j][:, :], zsrc[:, :])
            nc.scalar.copy(hbc[j][:, :], hbz[j][:, :])
        nc.gpsimd.ap_gather(
            wsel[:, :].rearrange("p (n d) -> p n d", d=1),
            wtab2[:, :].rearrange("p (n d) -> p n d", d=1),
            idx[:, :],
            channels=128, num_elems=256, d=1, num_idxs=D,
        )

        # bulk loads after the gather chain
        nc.sync.dma_start(wqs[:, :], wq_d[:, :])
        nc.scalar.dma_start(ones32[:, :], ones32_d[:, :])
        for hq in range(8):
            eng = (nc.sync, nc.scalar)[hq % 2]
            n = 2 * IMG_PAD if hq % 2 == 0 else IMG_PAD
            eng.dma_start(patch[16 * hq:16 * hq + 10, 0:n],
                          taps_d[16 * hq:16 * hq + 10, 0:n])
            # zero the 6-row gap (read by the 26-partition gather-dot)
            eng.dma_start(patch[16 * hq + 10:16 * hq + 16, 0:IMG_PAD],
                          taps_d[16 * hq + 10:16 * hq + 16, 0:IMG_PAD])

        # gathered-term dot: all 8 images at once. Partition rows 0..121
        # hold every image's tap rows in the same free layout (rows 32q+t
        # and 32q+16+t; the 6-row gaps are zero), matching wsel exactly, so
        # one fused multiply+reduce covers everything. VE time scales with
        # the free size only, so this costs the same as a single image pair.
        pv = patch[0:122, 0:IMG_PAD].rearrange(
            "p (r c) -> p r c", c=PW)[:, 1:65, 1:65]
        wv = wsel[0:122, :].rearrange("p (r c) -> p r c", c=Wd)
        nc.vector.scalar_tensor_tensor(
            selprod[0:122, :].rearrange("p (r c) -> p r c", c=Wd),
            pv, 1.0, wv,
            ALU.mult, ALU.mult,
            accum_out=acc_hsel[0:122, 0:1],
        )

        # ---------------- main pipeline: 2 waves x 4 images ----------------
        # (img g, quarter s) -> col-group c = g//2, partition 32c+4*(g%2)+s.
        # Images run 4 at a time (one per 32-partition tap strip): conv MMs
        # occupy all four PE row-groups, indicator MMs all four col-groups.
        # Indicator MMs are deferred a tile so the PE never waits in-order.
        se = seps.tile([128, 1024], f32, tag="se")
        pending = []     # (g, s, sub, slot, eh, hh)
        region_cnt = {}  # (col-group, sub-slice) -> MMs seen; 16 each

        def flush_pending():
            for g, s, sub, slot, eh, hh in pending:
                c = g // 2
                b = 4 * g + s
                k = region_cnt.get((c, sub), 0)
                region_cnt[(c, sub)] = k + 1
                nc.tensor.matmul(
                    se[32 * c:32 * c + 32, sub * 512:sub * 512 + 512],
                    ones32[:, 32 * b:32 * b + 32],
                    eh[:, slot * 512:(slot + 1) * 512],
                    start=(k == 0),
                    stop=(k == 15),
                    tile_position=(0, 32 * c),
                    skip_group_check=True,
                )
            pending.clear()

        for wave in range(2):            # imgs (0,2,4,6) then (1,3,5,7)
            imgs = [wave, 2 + wave, 4 + wave, 6 + wave]
            views = {}
            for g in imgs:
                off = (g % 2) * IMG_PAD
                views[g] = patch[32 * (g // 2):32 * (g // 2) + 10,
                                 off:off + IMG_PAD].rearrange(
                    "p (r c) -> p r c", c=PW)[:, 1:65, 1:65]
            for s in range(4):           # 1024-px quarters, imgs in lockstep
                for hh in range(2):
                    for pair in range(2):
                        ga, gb = imgs[2 * pair], imgs[2 * pair + 1]
                        for sub in range(2):
                            hp = hps.tile([128, 1024], f32, tag="h")
                            r0 = s * 16 + sub * 8
                            nc.tensor.matmul(
                                hp[:, 0:512],
                                wqs[32 * (ga // 2):32 * (ga // 2) + 10,
                                    hh * 128:(hh + 1) * 128],
                                views[ga][:, r0:r0 + 8, :],
                                start=True, stop=True,
                                tile_position=(32 * (ga // 2), 0),
                            )
                            nc.tensor.matmul(
                                hp[:, 512:1024],
                                wqs[32 * (gb // 2):32 * (gb // 2) + 10,
                                    hh * 128:(hh + 1) * 128],
                                views[gb][:, r0:r0 + 8, :],
                                start=True, stop=True,
                                tile_position=(32 * (gb // 2), 0),
                            )
                            eh = expp.tile([128, 1024], bf16, tag="eh")
                            nc.scalar.activation(eh[:, :], hp[:, :], AF.Exp)
                            pending.append((ga, s, sub, 0, eh, hh))
                            pending.append((gb, s, sub, 1, eh, hh))
                            if len(pending) >= 8:
                                flush_pending()
        flush_pending()

        # ---------------- LSE: Ln + accumulate straight from PSUM ----------
        for c in range(4):
            nc.scalar.activation(lse_s[32 * c:32 * c + 8, :],
                                 se[32 * c:32 * c + 8, :], AF.Ln,
                                 accum_out=acc_lse[32 * c:32 * c + 8, 0:1])

        # ---------------- final combine ----------------
        # loss_partial = sum(acc_lse) - sum(acc_hsel), accumulated in one
        # PSUM cell via a +1 column and a -1 column.
        nc.tensor.matmul(se[0:1, 0:1], onescol[:, 0:1],
                         acc_lse[:, 0:1], start=True, stop=False,
                         tile_position=(0, 0), skip_group_check=True)
        nc.tensor.matmul(se[0:1, 0:1], negcol[:, 0:1],
                         acc_hsel[:, :], start=False, stop=True,
                         tile_position=(0, 0), skip_group_check=True)
        nc.vector.tensor_copy(fin[0:1, 0:1], se[0:1, 0:1])
        nc.sync.dma_start(out_d[:, :], fin[0:1, 0:1])

    nc.finalize()
    return nc


def _host_inputs(x, W, b):
    """Per-core input maps (host-side re-layout only, no arithmetic on x
    beyond dtype conversion)."""
    x = np.ascontiguousarray(np.asarray(x, dtype=np.float32).reshape(64, H, Wd))
    W = np.asarray(W, dtype=np.float32).reshape(K, 3, 3)
    b = np.asarray(b, dtype=np.float32)

    # conv lhsT replicas: strip 32Q+t rows, col block hh -> W[t, class]
    wq = np.zeros((128, 256), dtype=BF16)
    wtab = np.zeros((128, 256), dtype=np.float32)
    for t, (dy, dx) in enumerate(TAPS):
        wrow = W[:, 1 + dy, 1 + dx]
        for q in range(4):
            wq[32 * q + t, :] = wrow.astype(BF16)
        for g in range(8):
            wtab[16 * g + t, :] = wrow
    for q in range(4):
        wq[32 * q + 9, :] = b.astype(BF16)
    for g in range(8):
        wtab[16 * g + 9, :] = b

    ones32 = np.zeros((128, 1024), dtype=BF16)
    for g in range(8):
        for sq in range(4):
            bb = 4 * g + sq
            local = 4 * (g % 2) + sq
            ones32[:, 32 * bb + local] = BF16(1.0)

    in_maps = []
    for c in range(N_CORES):
        xs = x[c * IMGS:(c + 1) * IMGS]                      # (8, 64, 64) f32
        xb = xs.astype(BF16)
        shifted = np.zeros((IMGS, 10, PW, PW), dtype=BF16)   # [img, tap, 66, 66]
        for t, (dy, dx) in enumerate(TAPS):
            r0, r1 = max(0, dy), min(H, H + dy)
            c0, c1 = max(0, dx), min(Wd, Wd + dx)
            dr0 = 1 + (r0 - dy)
            dc0 = 1 + (c0 - dx)
            shifted[:, t, dr0:dr0 + (r1 - r0), dc0:dc0 + (c1 - c0)] = \
                xb[:, r0:r1, c0:c1]
        shifted[:, 9, 1:65, 1:65] = BF16(1.0)
        taps = np.zeros((128, 2 * IMG_PAD), dtype=BF16)
        for qq in range(4):
            for t in range(10):
                taps[32 * qq + t, 0:IMG_PAD] = shifted[2 * qq, t].reshape(-1)
                taps[32 * qq + t, IMG_PAD:] = shifted[2 * qq + 1, t].reshape(-1)
                taps[32 * qq + 16 + t, 0:IMG_PAD] = \
                    shifted[2 * qq + 1, t].reshape(-1)
        xw = np.ascontiguousarray(
            xs.reshape(IMGS, D // 16, 16).transpose(0, 2, 1).reshape(128, D // 16)
        ).astype(np.float32)
        in_maps.append({
            "taps": taps,
            "xw": xw,
            "wq": wq,
            "wtab": wtab,
            "ones32": ones32,
        })
    return in_maps


def kernel(x, W, b):
    from concourse.bass_utils import run_bass_kernel_spmd

    key = "main"
    if key not in _COMPILED:
        _COMPILED[key] = _build_nc(idx_shift=-0.5)
    nc = _COMPILED[key]
    in_maps = _host_inputs(x, W, b)
    res = run_bass_kernel_spmd(nc, in_maps, core_ids=list(range(N_CORES)))
    total = np.float64(0.0)
    for r in res.results:
        total += np.float64(r["out"].reshape(-1)[0])
    return np.float32(total / 64.0)
